# revision 24
# baseline (speedup 1.0000x reference)
"""ECE loss (equal-width 15-bin) for [1048576, 128] logits on 8 TRN2 NeuronCores.

Strategy (data-parallel over rows):
  Host first converts y_pred to fp16 (monotone rounding, replicable on
  host for the accuracy-equality test). This halves HBM traffic (32MB
  per core) and unlocks the DVE 2x mode for tensor_tensor ops.

  Device, per core (N/8 = 131072 rows as [128 partitions x 1024 slots]):
    - stream [128, G, 128] fp16 supertiles of x
    - DVE max:  TT-max tree w64->w32->w16 (all-fp16 TT runs at 2x,
      ~0.52ns/out-elem) then reduce_max fp16 [.,16] -> m16.
      max of fp16 values == fp16-monotone max; host reproduces it
      exactly from its own fp16 conversion.
    - ACT: e = exp(x) -> fp16 (batched) for most slots; the first KA
      slots of each supertile instead use ACT's fused exp+accum_out
      (f32 row-sum in one [128,1,C] instruction), offloading sum work
      from the saturated DVE to ACT's idle capacity.
    - DVE sum:  TT-add tree on e -> u16 for the batched slots.
  Host:
    conf = exp(m16)/u; acc = (x16[r, y_true[r]] == m16); then the
    15-bin histogram + ECE as in the reference.

Both engines run balanced at ~152ns per 128-elem row-slot (DVE: TT
trees at the 2x fp16 rate + short reduces; ACT: exp at 1 elem/cycle
plus the accum rows). TENSOR_REDUCE has no fp16 fast mode on TRN2,
which is why the trees are TT-based. Warm-up/warm-down supertile
sizes bound the DMA-latency ramp and the drain tail; the slim exit
barrier and memset-provided exp bias trim the framework pre/postamble.

Measured: 271.1us (f32 all-reduce baseline) -> ~177.8us; relative
error vs the exact f32 reference path ~1.8e-4 (tolerance 2e-2).
"""

import numpy as np

import concourse.bacc as bacc
import concourse.tile as tile
from concourse import mybir
from concourse.bass_utils import run_bass_kernel_spmd

N_CORES = 8
N = 1048576
C = 128
N_SHARD = N // N_CORES  # 131072
P = 128                 # SBUF partitions
T = N_SHARD // P        # 1024 row-slots per partition
N_BINS = 15

# supertile schedule: warm-up (fast start) and warm-down (short tail).
GS = [8, 8, 16, 16, 32] + [96] * 9 + [48, 16, 16]
assert sum(GS) == T
KA_PER_64 = 5  # accum slots per 64 (fused exp+sum on ACT)
SCHED = []
_t0 = 0
for _g in GS:
    SCHED.append((_t0, _g, (_g * KA_PER_64) // 64))
    _t0 += _g
NA = sum(ka for _, _, ka in SCHED)  # accum slots per partition

_CACHE: dict = {}


class _SlimTileContext(tile.TileContext):
    """TileContext with a cheaper exit: the post-clear barrier is sem-only."""

    def _drain_and_barrier(self, tick_clock, wait_clock):
        drain_inst = self.nc.sync.drain()
        wait_clock.add_sem_waits(
            drain_inst.ins, tile.ScopedClock({None: tick_clock.global_clock})
        )
        self.nc.all_engine_barrier()
        assert self.sems is not None
        popped = self.nc._tile_sem_poison_stack.pop()
        assert popped is self._sem_poison
        self.nc.clear_and_free_semaphores(list(self.sems.allocated().values()))
        self.nc.all_engine_barrier(sem_only=True)


def _build_bass():
    nc = bacc.Bacc(None, target_bir_lowering=False)
    x = nc.dram_tensor("x", [N_SHARD, C], mybir.dt.float16, kind="ExternalInput")
    m_out = nc.dram_tensor("m_out", [N_SHARD], mybir.dt.float16, kind="ExternalOutput")
    u_out = nc.dram_tensor("u_out", [N_SHARD], mybir.dt.float16, kind="ExternalOutput")
    ua_out = nc.dram_tensor("ua_out", [P * NA], mybir.dt.float32, kind="ExternalOutput")

    xv = x[:, :].rearrange("(p t) c -> p t c", p=P)
    mv = m_out[:].rearrange("(p t) -> p t", p=P)
    uv = u_out[:].rearrange("(p t) -> p t", p=P)
    uav = ua_out[:].rearrange("(p t) -> p t", p=P)

    with _SlimTileContext(nc) as tc:
        with (
            tc.tile_pool(name="xin", bufs=4) as xin_pool,
            tc.tile_pool(name="exps", bufs=2) as exp_pool,
            tc.tile_pool(name="tree", bufs=2) as tree_pool,
            tc.tile_pool(name="stats", bufs=1) as stats_pool,
        ):
            m_all = stats_pool.tile([P, T], mybir.dt.float16)
            u_all = stats_pool.tile([P, T], mybir.dt.float16)
            ua_all = stats_pool.tile([P, max(NA, 1)], mybir.dt.float32)
            # zero bias tile for Exp: avoids the framework const-tensor
            # TENSOR_LOADs in the startup critical path
            bias0 = stats_pool.tile([P, 1], mybir.dt.float32)
            nc.vector.memset(bias0[:], 0.0)
            a_off = 0
            flushed = 0
            a_flushed = 0
            for si, (t0, g, ka) in enumerate(SCHED):
                kb = g - ka
                xt = xin_pool.tile([P, g, C], mybir.dt.float16, tag="xt")
                nc.sync.dma_start(out=xt[:], in_=xv[:, t0 : t0 + g, :])

                # --- max: fp16 TT tree (2x mode) + short reduce, all slots
                h1 = tree_pool.tile([P, g, 64], mybir.dt.float16, tag="h1")
                nc.vector.tensor_tensor(
                    out=h1[:], in0=xt[:, :, 0:64], in1=xt[:, :, 64:128],
                    op=mybir.AluOpType.max,
                )
                h2 = tree_pool.tile([P, g, 32], mybir.dt.float16, tag="h2")
                nc.vector.tensor_tensor(
                    out=h2[:], in0=h1[:, :, 0:32], in1=h1[:, :, 32:64],
                    op=mybir.AluOpType.max,
                )
                h3 = tree_pool.tile([P, g, 16], mybir.dt.float16, tag="h3")
                nc.vector.tensor_tensor(
                    out=h3[:], in0=h2[:, :, 0:16], in1=h2[:, :, 16:32],
                    op=mybir.AluOpType.max,
                )
                h4 = tree_pool.tile([P, g, 8], mybir.dt.float16, tag="h4")
                nc.vector.tensor_tensor(
                    out=h4[:], in0=h3[:, :, 0:8], in1=h3[:, :, 8:16],
                    op=mybir.AluOpType.max,
                )
                nc.vector.reduce_max(
                    out=m_all[:, t0 : t0 + g], in_=h4[:], axis=mybir.AxisListType.X
                )

                # --- batched exp on ACT (fp16 in/out), slots [ka:g]
                et = exp_pool.tile([P, kb, C], mybir.dt.float16, tag="et")
                nc.scalar.activation(
                    out=et[:], in_=xt[:, ka:g, :], func=mybir.ActivationFunctionType.Exp,
                    bias=bias0[:],
                )
                # --- accum slots [0:ka]: fused exp + f32 row-sum on ACT
                for j in range(ka):
                    esc = exp_pool.tile([P, 1, C], mybir.dt.float16, tag="esc")
                    nc.scalar.activation(
                        out=esc[:], in_=xt[:, j : j + 1, :],
                        func=mybir.ActivationFunctionType.Exp,
                        bias=bias0[:],
                        accum_out=ua_all[:, a_off + j : a_off + j + 1],
                    )

                # --- sum: fp16 TT tree (2x) + short reduce, slots [ka:g]
                s1 = tree_pool.tile([P, kb, 64], mybir.dt.float16, tag="h1")
                nc.vector.tensor_tensor(
                    out=s1[:], in0=et[:, :, 0:64], in1=et[:, :, 64:128],
                    op=mybir.AluOpType.add,
                )
                s2 = tree_pool.tile([P, kb, 32], mybir.dt.float16, tag="h2")
                nc.vector.tensor_tensor(
                    out=s2[:], in0=s1[:, :, 0:32], in1=s1[:, :, 32:64],
                    op=mybir.AluOpType.add,
                )
                s3 = tree_pool.tile([P, kb, 16], mybir.dt.float16, tag="h3")
                nc.vector.tensor_tensor(
                    out=s3[:], in0=s2[:, :, 0:16], in1=s2[:, :, 16:32],
                    op=mybir.AluOpType.add,
                )
                s4 = tree_pool.tile([P, kb, 8], mybir.dt.float16, tag="h4")
                nc.vector.tensor_tensor(
                    out=s4[:], in0=s3[:, :, 0:8], in1=s3[:, :, 8:16],
                    op=mybir.AluOpType.add,
                )
                with nc.allow_low_precision("fp16 sum-of-exp; validated 2e-4"):
                    nc.vector.reduce_sum(
                        out=u_all[:, t0 + ka : t0 + g], in_=s4[:],
                        axis=mybir.AxisListType.X,
                    )

                a_off += ka
                if si % 4 == 3 or si >= len(SCHED) - 3:
                    t1 = t0 + g
                    nc.sync.dma_start(out=mv[:, flushed:t1], in_=m_all[:, flushed:t1])
                    nc.sync.dma_start(out=uv[:, flushed:t1], in_=u_all[:, flushed:t1])
                    if a_off > a_flushed:
                        nc.sync.dma_start(
                            out=uav[:, a_flushed:a_off], in_=ua_all[:, a_flushed:a_off]
                        )
                    flushed = t1
                    a_flushed = a_off
    nc.finalize()
    return nc


def run_device(y_pred: np.ndarray, **spmd_kwargs):
    """Run the bass kernel on 8 cores; returns (m16, u) with u merged f64."""
    if "nc" not in _CACHE:
        _CACHE["nc"] = _build_bass()
    nc = _CACHE["nc"]
    x16 = y_pred if y_pred.dtype == np.float16 else y_pred.astype(np.float16)
    in_maps = [{"x": x16[c * N_SHARD : (c + 1) * N_SHARD]} for c in range(N_CORES)]
    res = run_bass_kernel_spmd(nc, in_maps, core_ids=list(range(N_CORES)), **spmd_kwargs)
    m = np.concatenate([r["m_out"] for r in res.results])
    u_parts = []
    for r in res.results:
        u = r["u_out"].reshape(P, T).astype(np.float64)
        ua = r["ua_out"].reshape(P, NA)
        a_off = 0
        for t0, g, ka in SCHED:
            u[:, t0 : t0 + ka] = ua[:, a_off : a_off + ka]
            a_off += ka
        u_parts.append(u.reshape(P * T))
    u = np.concatenate(u_parts)
    return m, u, res


def finish_host(x16, y_true, m16, u) -> np.ndarray:
    xl = x16[np.arange(N), np.asarray(y_true, dtype=np.int64)]
    conf = np.exp(m16.astype(np.float64)) / u
    acc = (xl == m16).astype(np.float64)
    bin_idx = np.clip(np.ceil(conf * N_BINS).astype(np.int64) - 1, 0, N_BINS - 1)
    cnt = np.bincount(bin_idx, minlength=N_BINS).astype(np.float64)
    conf_sum = np.bincount(bin_idx, weights=conf, minlength=N_BINS)
    acc_sum = np.bincount(bin_idx, weights=acc, minlength=N_BINS)
    safe = np.where(cnt > 0, cnt, 1.0)
    per_bin = np.where(cnt > 0, np.abs(conf_sum / safe - acc_sum / safe) * (cnt / N), 0.0)
    return np.array([per_bin.sum()], dtype=np.float32)


def kernel(y_pred: np.ndarray, y_true: np.ndarray) -> np.ndarray:
    x16 = np.ascontiguousarray(np.asarray(y_pred, dtype=np.float32)).astype(np.float16)
    m16, u, _ = run_device(x16)
    return finish_host(x16, y_true, m16, u)
